# revision 4
# baseline (speedup 1.0000x reference)
"""DTM decoder kernel for one TRN2 chip (8 NeuronCores), tensor-parallel
over the vocab dimension.

Math (reference):
    logits[t,k,v] = sum_e topic_emb[t,k,e] * word_emb[v,e]        (T*K=500, V=50000)
    betas = softmax(logits, axis=v)
    out[b,:] = theta[b,:] @ betas[time_index[b]]                  (B=256)

Parallelization: shard V across 8 cores (V_c = 6250). Each core, flash-style:
  1. matmul1 per (tk-tile, v-chunk): logits chunk in PSUM (f32 accum over E),
     DVE chunk row-max (negated), ScalarE evicts PSUM with exp(l - m_chunk)
     into the persistent P tiles and accumulates the chunk row-sum.
     The exp runs concurrently with the remaining matmuls.
  2. tiny per-tile combines give local stats (m_c, s_c); a 4KB AllGather
     shares them; each core recomputes global (m_g, s_g).
  3. matmul2 per v-chunk j: theta'_j[tk,b] = theta[tk,b]*exp(m_chunk_j - m_g)/s_g
     (per-chunk scale absorbs both the flash rescale and the softmax
     normalization), out_chunk = theta'_j^T @ P_j.

Perf notes (vs the first working version):
  - Head: topic and the first wemb slab are loaded with per-e-chunk 2D DMAs
    issued from TWO engines (sync: w0, vector: topic) so HWDGE descriptor
    issue (~0.65us per plane, serialized per engine) overlaps, and the PE can
    start accumulating e-chunk 0 while later chunks are still in flight.
  - The batch is sorted by time_index on host. Sorted, batch column block 0
    (cols 0..127) only needs tk-tiles 0..T0 and block 1 needs tiles T1..3,
    so matmul2 drops from 8 to (T0+1)+(4-T1) (typically 5) matmuls per
    v-chunk. (T0, T1) are derived from the actual time_index at runtime and
    the NEFF is compiled lazily per (T0, T1) -- worst case (3, 0) is the
    dense fallback, so any input distribution is handled correctly.
  - V chunks ordered [310, 512*11, 308]: a smaller first slab starts the PE
    earlier; a smaller last chunk shrinks the mm2 drain tail.

Host side: word_embeddings is passed per-core pre-transposed ([E, V_c]) so the
contraction dim lands on SBUF partitions with no on-device transpose;
time_index gather is folded into a (TK, B) theta matrix on host (tiny).
Matmuls run as float32r (fp32 storage, reduced-precision multiply; measured
0.54 ns/col on HW); set DTM_MM1/DTM_MM2=f32 for exact-but-4x-slower.
"""

import os
import sys

if "/opt/trn_rl_repo" not in sys.path:
    sys.path.insert(0, "/opt/trn_rl_repo")

import numpy as np

from concourse import bacc, mybir, tile
from concourse.masks import make_identity
from concourse.bass_utils import run_bass_kernel_spmd

B, V, K, T, E = 256, 50000, 50, 10, 1024
TK = T * K  # 500
N_CORES = 8
VC = V // N_CORES  # 6250
P = 128

TK_CHUNKS = [(0, 128), (128, 128), (256, 128), (384, 116)]
E_CHUNKS = 8  # E / 128
# All chunks >= 256 (float32r full rate) and even (fp32r ISA restriction).
# Small chunks first (earlier PE start) and last (shorter drain tail).
V_CHUNKS = [(0, 310)] + [(310 + i * 512, 512) for i in range(11)] + [(5942, 308)]
assert sum(n for _, n in V_CHUNKS) == VC

F32 = mybir.dt.float32
Exp = mybir.ActivationFunctionType.Exp

_MM1_DT = {"f32": F32, "f32r": mybir.dt.float32r}[os.environ.get("DTM_MM1", "f32r")]
_MM2_DT = {"f32": F32, "f32r": mybir.dt.float32r, "bf16": mybir.dt.bfloat16}[
    os.environ.get("DTM_MM2", "f32r")
]


def build(bt0=3, bt1=0, vc=VC, v_chunks=None, debug=False):
    """bt0: last tk-tile needed by batch cols 0..127; bt1: first tile needed
    by cols 128..255 (batch sorted by time on host). (3, 0) is dense."""
    if v_chunks is None:
        v_chunks = V_CHUNKS
    nvc = len(v_chunks)
    blk_tiles = [list(range(0, bt0 + 1)), list(range(bt1, 4))]
    used_tiles = sorted(set(blk_tiles[0]) | set(blk_tiles[1]))
    nc = bacc.Bacc("TRN2", target_bir_lowering=False, debug=debug, num_devices=N_CORES)

    wembT = nc.dram_tensor("wembT", [E, vc], _MM1_DT, kind="ExternalInput").ap()
    topicT = nc.dram_tensor("topicT", [E, TK], _MM1_DT, kind="ExternalInput").ap()
    thetaT = nc.dram_tensor("thetaT", [TK, B], F32, kind="ExternalInput").ap()
    out = nc.dram_tensor("out", [B, vc], F32, kind="ExternalOutput").ap()

    # stats layout: [0:512] row-max m_c, [512:1024] row-sum s_c (500 used)
    stats_local = nc.dram_tensor("stats_local", [1, 1024], F32)
    stats_all = nc.dram_tensor("stats_all", [N_CORES, 1024], F32, addr_space="Shared")
    dummy_in = nc.dram_tensor("dummy_in", [1, 16], F32)
    dummy_all = nc.dram_tensor("dummy_all", [N_CORES, 16], F32, addr_space="Shared")

    with tile.TileContext(nc) as tc:
        with (
            tc.tile_pool(name="pbig", bufs=1) as pbig,
            tc.tile_pool(name="const", bufs=1) as const,
            tc.tile_pool(name="wpool", bufs=3) as wpool,
            tc.tile_pool(name="thp", bufs=8) as thp,
            tc.tile_pool(name="opool", bufs=3) as opool,
            tc.tile_pool(name="psp", bufs=4, space="PSUM") as psp,
        ):
            # --- head: first loads, issued from two engines in parallel ---
            # slab[p, e, v] = wembT[e*128 + p, v]; per-e 2D pieces so matmul
            # e-accumulation starts as soon as piece e lands.
            v0_0, nv_0 = v_chunks[0]
            w0 = wpool.tile([P, E_CHUNKS, 512], _MM1_DT, tag="w", name="w0")
            for e in range(E_CHUNKS):
                nc.sync.dma_start(
                    out=w0[:, e, :nv_0],
                    in_=wembT[e * P : (e + 1) * P, v0_0 : v0_0 + nv_0],
                )
            # preload the exp table set on ScalarE while the first DMAs run
            warm = const.tile([P, 2], F32, tag="warm", name="warm")
            nc.vector.memset(warm[:], 0.0)
            nc.scalar.activation(warm[:], warm[:], Exp)
            # tiny throwaway AllGather: pays the ncfw/NCCL first-call setup
            # early, overlapped with matmul1, so the real one is cheaper
            dz = const.tile([1, 16], F32, tag="dz", name="dz")
            nc.gpsimd.memset(dz[:], 0.0)
            nc.gpsimd.dma_start(out=dummy_in[:], in_=dz[:])
            nc.gpsimd.collective_compute(
                "AllGather",
                mybir.AluOpType.bypass,
                replica_groups=[list(range(N_CORES))],
                ins=[dummy_in[:].opt()],
                outs=[dummy_all[:].opt()],
            )
            # topic[p, e, t] = topicT[e*128 + p, t] -- issued from gpsimd
            # (SWDGE) so it overlaps the sync engine's w0 descriptor issue.
            # NOT on the scalar engine: HWDGE issue from Activation degrades
            # the loaded activation (exp) table -> ~1e-2 output error.
            topic_sb = const.tile([P, E_CHUNKS, TK], _MM1_DT, tag="topic", name="topic")
            for e in range(E_CHUNKS):
                nc.gpsimd.dma_start(
                    out=topic_sb[:, e, :], in_=topicT[e * P : (e + 1) * P, :]
                )
            # identity for the phase-2/3 PE transposes (gpsimd; needed ~late)
            ident = const.tile([P, P], F32, tag="ident", name="ident")
            make_identity(nc, ident[:])

            # theta_all[p, i, b] = thetaT[i*128 + p, b] (i*128+p < 500)
            theta_all = const.tile([P, 4, B], F32, tag="theta", name="theta")
            theta_sb = [theta_all[:, i, :] for i in range(4)]
            # msall[p, i, j]: j=0 -> m_c, j=1 -> s_c for tk-tile i
            msall = const.tile([P, 4, 2], F32, tag="msall", name="msall")
            mrun = [msall[:, i, 0:1] for i in range(4)]
            sloc = [msall[:, i, 1:2] for i in range(4)]
            p_t, negmm, smat = [], [], []
            for i, (r0, rows) in enumerate(TK_CHUNKS):
                p_t.append(pbig.tile([P, vc], _MM2_DT, tag=f"P{i}", name=f"P{i}"))
                nm = const.tile([P, nvc], F32, tag=f"negmm{i}", name=f"negmm{i}")
                negmm.append(nm)
                sm = const.tile([P, nvc], F32, tag=f"smat{i}", name=f"smat{i}")
                smat.append(sm)

            # --- phase 1: logits chunks; fused exp-evict (flash style) ---
            for vi, (v0, nv) in enumerate(v_chunks):
                if vi == 0:
                    wt = w0
                else:
                    wt = wpool.tile([P, E_CHUNKS, 512], _MM1_DT, tag="w", name="w")
                    nc.sync.dma_start(
                        out=wt[:, :, :nv],
                        in_=wembT[:, v0 : v0 + nv].rearrange(
                            "(e p) v -> p e v", e=E_CHUNKS, p=P
                        ),
                    )
                for i, (r0, rows) in enumerate(TK_CHUNKS):
                    ps = psp.tile([P, 512], F32, tag="ps1", name="ps1", bufs=4)
                    for e in range(E_CHUNKS):
                        nc.tensor.matmul(
                            ps[:rows, :nv],
                            lhsT=topic_sb[:, e, r0 : r0 + rows],
                            rhs=wt[:, e, :nv],
                            start=(e == 0),
                            stop=(e == E_CHUNKS - 1),
                        )
                    # -chunk_rowmax (DVE), then exp-evict + chunk rowsum (ScalarE)
                    nc.vector.reduce_max(
                        negmm[i][:rows, vi : vi + 1],
                        ps[:rows, :nv],
                        axis=mybir.AxisListType.X,
                        negate=True,
                    )
                    nc.scalar.activation(
                        p_t[i][:rows, v0 : v0 + nv],
                        ps[:rows, :nv],
                        Exp,
                        bias=negmm[i][:rows, vi : vi + 1],
                        accum_out=smat[i][:rows, vi : vi + 1],
                    )

            # theta loads (phase-4 only; emitted late so startup DMA bandwidth
            # goes to topic + the first wemb slabs)
            nc.sync.dma_start(out=theta_all[:116, 3, :], in_=thetaT[384:500, :])
            nc.sync.dma_start(
                out=theta_all[:, 0:3, :],
                in_=thetaT[0:384].rearrange("(i p) b -> p i b", i=3, p=P),
            )
            # padded stat rows: m_c = 0, s_c = 1 (emitted here, used by the
            # phase-2 transpose; DVE program order keeps them before the
            # per-tile stat writes below)
            nc.vector.memset(msall[:, :, 0:1], 0.0)
            nc.vector.memset(msall[:, :, 1:2], 1.0)

            # --- phase 2: local stats + allgather ---
            for i, (r0, rows) in enumerate(TK_CHUNKS):
                # m_c = max_j m_j = -(min_j negm_j)
                nc.vector.tensor_reduce(
                    out=mrun[i][:rows, :],
                    in_=negmm[i][:rows, :nvc],
                    op=mybir.AluOpType.min,
                    axis=mybir.AxisListType.X,
                    negate=True,
                )
                nmc = const.tile([P, 1], F32, tag=f"nmc{i}", name=f"nmc{i}")
                nc.vector.tensor_scalar_mul(nmc[:rows, :], mrun[i][:rows, :], -1.0)
                # s_c = sum_j s_j * exp(m_j - m_c);  m_j = -negmm[:, j]
                wj = const.tile([P, nvc], F32, tag=f"wj{i}", name=f"wj{i}")
                nc.scalar.activation(
                    wj[:rows, :nvc],
                    negmm[i][:rows, :nvc],
                    Exp,
                    bias=nmc[:rows, :],
                    scale=-1.0,
                )
                nc.vector.tensor_mul(
                    wj[:rows, :nvc], wj[:rows, :nvc], smat[i][:rows, :nvc]
                )
                nc.vector.reduce_sum(
                    sloc[i][:rows, :], wj[:rows, :nvc], axis=mybir.AxisListType.X
                )
            # transpose [128, 8] -> [8, 128] on the (idle) PE so the stats DMA
            # is 8 contiguous 512B runs instead of a 4B-granular scatter
            mst_ps = psp.tile([8, P], F32, tag="ps2", name="mst_ps", bufs=4)
            nc.tensor.transpose(mst_ps[:], msall[:].rearrange("p i j -> p (i j)"), ident[:])
            msT = const.tile([8, P], F32, tag="msT", name="msT")
            nc.vector.tensor_copy(msT[:], mst_ps[:])
            # stats_local[0, (i*2+j)*128 + p] = m/s[tile i, row p]
            nc.sync.dma_start(
                out=stats_local[0].rearrange("(q p) -> q p", q=8, p=P), in_=msT[:]
            )
            nc.gpsimd.collective_compute(
                "AllGather",
                mybir.AluOpType.bypass,
                replica_groups=[list(range(N_CORES))],
                ins=[stats_local[:].opt()],
                outs=[stats_all[:].opt()],
            )

            # --- phase 3: global stats; per-chunk scale matrix G ---
            # natural-layout gather (8 contiguous 4KB runs), then PE-transpose
            # each [8, 128] block to the [tk-row, core] layout the combines need
            sg_all = const.tile([8, 2 * 4 * P], F32, tag="sg_all", name="sg_all")
            nc.sync.dma_start(out=sg_all[:], in_=stats_all[:])
            mst = const.tile([P, 4, 2, N_CORES], F32, tag="mst", name="mst")
            for q in range(8):
                i, j = q // 2, q % 2
                if i not in used_tiles:
                    continue
                tp = psp.tile([P, 8], F32, tag="ps2", name="mst_ps2", bufs=4)
                nc.tensor.transpose(
                    tp[:], sg_all[:, q * P : (q + 1) * P], ident[0:8, 0:8]
                )
                nc.vector.tensor_copy(mst[:, i, j, :], tp[:])
            gmat = {}
            for i in used_tiles:
                r0, rows = TK_CHUNKS[i]
                mt = mst[:, i, 0, :]
                st = mst[:, i, 1, :]
                nmg = const.tile([P, 1], F32, tag=f"nmg{i}", name=f"nmg{i}")
                nc.vector.reduce_max(
                    nmg[:], mt[:], axis=mybir.AxisListType.X, negate=True
                )
                wt = const.tile([P, N_CORES], F32, tag=f"wt{i}", name=f"wt{i}")
                nc.scalar.activation(wt[:], mt[:], Exp, bias=nmg[:])
                nc.vector.tensor_mul(wt[:], wt[:], st[:])
                sg = const.tile([P, 1], F32, tag=f"sg{i}", name=f"sg{i}")
                nc.vector.reduce_sum(sg[:], wt[:], axis=mybir.AxisListType.X)
                rg = const.tile([P, 1], F32, tag=f"rg{i}", name=f"rg{i}")
                nc.vector.reciprocal(rg[:], sg[:])
                # G[:, j] = exp(m_j - m_g) / s_g  (m_j = -negmm[:, j])
                g = const.tile([P, nvc], F32, tag=f"g{i}", name=f"g{i}")
                nc.scalar.activation(
                    g[:rows, :nvc],
                    negmm[i][:rows, :nvc],
                    Exp,
                    bias=nmg[:rows, :],
                    scale=-1.0,
                )
                nc.vector.tensor_scalar_mul(
                    g[:rows, :nvc], g[:rows, :nvc], rg[:rows, :]
                )
                gmat[i] = g

            # --- phase 4: out[b, v_j] = sum_tk theta[tk,b]*G[tk,j] * P[tk,v_j]
            # batch sorted by time on host: col block 0 needs tiles 0..bt0,
            # block 1 needs tiles bt1..3 ---
            for vi, (v0, nv) in enumerate(v_chunks):
                thv = {}
                for i in used_tiles:
                    r0, rows = TK_CHUNKS[i]
                    tv = thp.tile([P, B], _MM2_DT, tag="thv", name="thv")
                    nc.vector.tensor_scalar_mul(
                        tv[:rows, :], theta_sb[i][:rows, :], gmat[i][:rows, vi : vi + 1]
                    )
                    thv[i] = tv
                ot = opool.tile([P, 2, 512], F32, tag="ot", name="ot")
                for bi, (b0, tl) in enumerate(zip((0, P), blk_tiles)):
                    ps = psp.tile([P, 512], F32, tag="ps2", name="ps2", bufs=4)
                    for idx, i in enumerate(tl):
                        r0, rows = TK_CHUNKS[i]
                        nc.tensor.matmul(
                            ps[:, :nv],
                            lhsT=thv[i][:rows, b0 : b0 + P],
                            rhs=p_t[i][:rows, v0 : v0 + nv],
                            start=(idx == 0),
                            stop=(idx == len(tl) - 1),
                        )
                    nc.scalar.copy(ot[:, bi, :nv], ps[:, :nv])
                nc.sync.dma_start(
                    out=out[:, v0 : v0 + nv].rearrange("(b p) v -> p b v", b=2, p=P),
                    in_=ot[:, :, :nv],
                )

    nc.compile()
    return nc


_NC_CACHE = {}


def _get_nc(bt0=3, bt1=0):
    key = (bt0, bt1)
    if key not in _NC_CACHE:
        _NC_CACHE[key] = build(bt0, bt1)
    return _NC_CACHE[key]


def _plan(ti):
    """Sort batch by time slice; derive which tk-tiles each column block of
    128 needs. Any distribution is handled (worst case = dense (3, 0))."""
    perm = np.argsort(ti, kind="stable")
    tis = ti[perm]
    bt0 = int((int(tis[P - 1]) * K + K - 1) // P)
    bt1 = int((int(tis[P]) * K) // P)
    return perm, tis, bt0, bt1


def prepare(theta, word_embeddings, topic_embeddings, time_index):
    """Host-side prep shared by kernel() and the profiling harness."""
    theta = np.ascontiguousarray(np.asarray(theta), dtype=np.float32)
    wemb = np.asarray(word_embeddings, dtype=np.float32)
    topic = np.asarray(topic_embeddings, dtype=np.float32)
    ti = np.asarray(time_index).astype(np.int64)

    perm, tis, bt0, bt1 = _plan(ti)

    # time-gathered theta for the SORTED batch, transposed:
    # thetaT[t*K + k, j] = theta[perm[j], k] iff tis[j] == t
    thetaT = np.zeros((TK, B), dtype=np.float32)
    rows = (tis[:, None] * K + np.arange(K)[None, :]).ravel()
    cols = np.repeat(np.arange(B), K)
    thetaT[rows, cols] = theta[perm].ravel()

    topicT = np.ascontiguousarray(topic.reshape(TK, E).T)  # [E, TK]

    in_maps = []
    for c in range(N_CORES):
        shard = np.ascontiguousarray(wemb[c * VC : (c + 1) * VC, :].T)  # [E, VC]
        in_maps.append({"wembT": shard, "topicT": topicT, "thetaT": thetaT})

    nc = _get_nc(bt0, bt1)
    return nc, in_maps, perm


def kernel(theta, word_embeddings, topic_embeddings, time_index):
    nc, in_maps, perm = prepare(theta, word_embeddings, topic_embeddings, time_index)
    res = run_bass_kernel_spmd(nc, in_maps, core_ids=list(range(N_CORES)))
    out_sorted = np.concatenate(
        [res.results[c]["out"] for c in range(N_CORES)], axis=1
    )
    out = np.empty_like(out_sorted)
    out[perm] = out_sorted
    return out


# revision 8
# speedup vs baseline: 1.0912x; 1.0912x over previous
"""DTM decoder kernel for one TRN2 chip (8 NeuronCores), tensor-parallel
over the vocab dimension.

Math (reference):
    logits[t,k,v] = sum_e topic_emb[t,k,e] * word_emb[v,e]        (T*K=500, V=50000)
    betas = softmax(logits, axis=v)
    out[b,:] = theta[b,:] @ betas[time_index[b]]                  (B=256)

Parallelization: shard V across 8 cores (V_c = 6250). Each core, flash-style:
  1. matmul1 per (tk-tile, v-chunk): logits chunk in PSUM (f32 accum over E),
     DVE chunk row-max (negated), ScalarE evicts PSUM with exp(l - m_chunk)
     into the persistent P tiles and accumulates the chunk row-sum.
     The exp runs concurrently with the remaining matmuls.
  2. tiny per-tile combines give local stats (m_c, s_c); a 4KB AllGather
     shares them; each core recomputes global (m_g, s_g).
  3. matmul2 per v-chunk j: theta'_j[tk,b] = theta[tk,b]*exp(m_chunk_j - m_g)/s_g
     (per-chunk scale absorbs both the flash rescale and the softmax
     normalization), out_chunk = theta'_j^T @ P_j.

Perf notes (vs the first working version):
  - Head: topic and the first wemb slab are loaded with per-e-chunk 2D DMAs
    issued from TWO engines (sync: w0, vector: topic) so HWDGE descriptor
    issue (~0.65us per plane, serialized per engine) overlaps, and the PE can
    start accumulating e-chunk 0 while later chunks are still in flight.
  - The batch is sorted by time_index on host. Sorted, batch column block 0
    (cols 0..127) only needs tk-tiles 0..T0 and block 1 needs tiles T1..3,
    so matmul2 drops from 8 to (T0+1)+(4-T1) (typically 5) matmuls per
    v-chunk. (T0, T1) are derived from the actual time_index at runtime and
    the NEFF is compiled lazily per (T0, T1) -- worst case (3, 0) is the
    dense fallback, so any input distribution is handled correctly.
  - V chunks ordered [310, 512*11, 308]: a smaller first slab starts the PE
    earlier; a smaller last chunk shrinks the mm2 drain tail.

Host side: word_embeddings is passed per-core pre-transposed ([E, V_c]) so the
contraction dim lands on SBUF partitions with no on-device transpose;
time_index gather is folded into a (TK, B) theta matrix on host (tiny).
Matmuls run as float32r (fp32 storage, reduced-precision multiply; measured
0.54 ns/col on HW); set DTM_MM1/DTM_MM2=f32 for exact-but-4x-slower.
"""

import os
import sys

if "/opt/trn_rl_repo" not in sys.path:
    sys.path.insert(0, "/opt/trn_rl_repo")

import numpy as np

from concourse import bacc, mybir, tile
from concourse.masks import make_identity
from concourse.bass_utils import run_bass_kernel_spmd

B, V, K, T, E = 256, 50000, 50, 10, 1024
TK = T * K  # 500
N_CORES = 8
VC = V // N_CORES  # 6250
P = 128

TK_CHUNKS = [(0, 128), (128, 128), (256, 128), (384, 116)]
E_CHUNKS = 8  # E / 128
# All chunks >= 256 (float32r full rate) and even (fp32r ISA restriction).
# Small chunks first (earlier PE start) and last (shorter drain tail).
V_CHUNKS = [(0, 310)] + [(310 + i * 512, 512) for i in range(11)] + [(5942, 308)]
assert sum(n for _, n in V_CHUNKS) == VC

F32 = mybir.dt.float32
Exp = mybir.ActivationFunctionType.Exp

_MM1_DT = {"f32": F32, "f32r": mybir.dt.float32r}[os.environ.get("DTM_MM1", "f32r")]
# bf16 for P/theta' halves the DVE thv work (2x 16-bit rate) and SBUF for P;
# emulated accuracy cost is ~25% on the final rel err (well inside 2e-2).
_MM2_DT = {"f32": F32, "f32r": mybir.dt.float32r, "bf16": mybir.dt.bfloat16}[
    os.environ.get("DTM_MM2", "bf16")
]


def build(bt0=3, bt1=0, vc=VC, v_chunks=None, debug=False):
    """bt0: last tk-tile needed by batch cols 0..127; bt1: first tile needed
    by cols 128..255 (batch sorted by time on host). (3, 0) is dense."""
    if v_chunks is None:
        v_chunks = V_CHUNKS
    nvc = len(v_chunks)
    blk_tiles = [list(range(0, bt0 + 1)), list(range(bt1, 4))]
    used_tiles = sorted(set(blk_tiles[0]) | set(blk_tiles[1]))
    nc = bacc.Bacc("TRN2", target_bir_lowering=False, debug=debug, num_devices=N_CORES)

    wembT = nc.dram_tensor("wembT", [E, vc], _MM1_DT, kind="ExternalInput").ap()
    topicT = nc.dram_tensor("topicT", [E, TK], _MM1_DT, kind="ExternalInput").ap()
    thetaT = nc.dram_tensor("thetaT", [TK, B], F32, kind="ExternalInput").ap()
    out = nc.dram_tensor("out", [B, vc], F32, kind="ExternalOutput").ap()

    # stats layout: [0:512] row-max m_c, [512:1024] row-sum s_c (500 used)
    stats_local = nc.dram_tensor("stats_local", [1, 1024], F32)
    stats_all = nc.dram_tensor("stats_all", [N_CORES, 1024], F32, addr_space="Shared")
    dummy_in = nc.dram_tensor("dummy_in", [1, 16], F32)
    dummy_all = nc.dram_tensor("dummy_all", [N_CORES, 16], F32, addr_space="Shared")

    with tile.TileContext(nc) as tc:
        with (
            tc.tile_pool(name="pbig", bufs=1) as pbig,
            tc.tile_pool(name="const", bufs=1) as const,
            tc.tile_pool(name="wpool", bufs=4) as wpool,
            tc.tile_pool(name="thp", bufs=8) as thp,
            tc.tile_pool(name="opool", bufs=3) as opool,
            tc.tile_pool(name="psp", bufs=4, space="PSUM") as psp,
        ):
            # --- head: first loads, issued from two engines in parallel ---
            # slab[p, e, v] = wembT[e*128 + p, v]; per-e 2D pieces so matmul
            # e-accumulation starts as soon as piece e lands.
            v0_0, nv_0 = v_chunks[0]
            w0 = wpool.tile([P, E_CHUNKS, 512], _MM1_DT, tag="w", name="w0")
            for e in range(4):
                nc.sync.dma_start(
                    out=w0[:, e, :nv_0],
                    in_=wembT[e * P : (e + 1) * P, v0_0 : v0_0 + nv_0],
                )
            # slab 1 issued mid-way through w0's pieces: its ~5us descriptor
            # issue overlaps the PE's first e-chunks so the chunk-1 matmuls
            # aren't DMA-starved during the ramp
            v0_1, nv_1 = v_chunks[1]
            w1 = wpool.tile([P, E_CHUNKS, 512], _MM1_DT, tag="w", name="w1")
            nc.sync.dma_start(
                out=w1[:, :, :nv_1],
                in_=wembT[:, v0_1 : v0_1 + nv_1].rearrange(
                    "(e p) v -> p e v", e=E_CHUNKS, p=P
                ),
            )
            for e in range(4, E_CHUNKS):
                nc.sync.dma_start(
                    out=w0[:, e, :nv_0],
                    in_=wembT[e * P : (e + 1) * P, v0_0 : v0_0 + nv_0],
                )
            # preload the exp table set on ScalarE while the first DMAs run
            warm = const.tile([P, 2], F32, tag="warm", name="warm")
            nc.vector.memset(warm[:], 0.0)
            nc.scalar.activation(warm[:], warm[:], Exp)
            # tiny throwaway AllGather: pays the ncfw/NCCL first-call setup
            # early, overlapped with matmul1, so the real one is cheaper
            dz = const.tile([1, 16], F32, tag="dz", name="dz")
            nc.gpsimd.memset(dz[:], 0.0)
            nc.gpsimd.dma_start(out=dummy_in[:], in_=dz[:])
            nc.gpsimd.collective_compute(
                "AllGather",
                mybir.AluOpType.bypass,
                replica_groups=[list(range(N_CORES))],
                ins=[dummy_in[:].opt()],
                outs=[dummy_all[:].opt()],
            )
            # topic[p, e, t] = topicT[e*128 + p, t] -- issued from gpsimd
            # (SWDGE) so it overlaps the sync engine's w0 descriptor issue.
            # NOT on the scalar engine: HWDGE issue from Activation degrades
            # the loaded activation (exp) table -> ~1e-2 output error.
            topic_sb = const.tile([P, E_CHUNKS, TK], _MM1_DT, tag="topic", name="topic")
            for e in range(E_CHUNKS):
                nc.gpsimd.dma_start(
                    out=topic_sb[:, e, :], in_=topicT[e * P : (e + 1) * P, :]
                )
            # identity for the phase-2/3 PE transposes (gpsimd; needed ~late)
            ident = const.tile([P, P], F32, tag="ident", name="ident")
            make_identity(nc, ident[:])

            # theta_all[p, i, b] = thetaT[i*128 + p, b] (i*128+p < 500)
            theta_all = const.tile([P, 4, B], F32, tag="theta", name="theta")
            theta_sb = [theta_all[:, i, :] for i in range(4)]
            # msall[p, i, j]: j=0 -> m_c, j=1 -> s_c for tk-tile i
            msall = const.tile([P, 4, 2], F32, tag="msall", name="msall")
            mrun = [msall[:, i, 0:1] for i in range(4)]
            sloc = [msall[:, i, 1:2] for i in range(4)]
            p_t, negmm, smat = [], [], []
            for i, (r0, rows) in enumerate(TK_CHUNKS):
                p_t.append(pbig.tile([P, vc], _MM2_DT, tag=f"P{i}", name=f"P{i}"))
                nm = const.tile([P, nvc], F32, tag=f"negmm{i}", name=f"negmm{i}")
                negmm.append(nm)
                sm = const.tile([P, nvc], F32, tag=f"smat{i}", name=f"smat{i}")
                smat.append(sm)

            # --- phase 1: logits chunks; fused exp-evict (flash style) ---
            for vi, (v0, nv) in enumerate(v_chunks):
                if vi == 0:
                    wt = w0
                elif vi == 1:
                    wt = w1
                else:
                    wt = wpool.tile([P, E_CHUNKS, 512], _MM1_DT, tag="w", name="w")
                    nc.sync.dma_start(
                        out=wt[:, :, :nv],
                        in_=wembT[:, v0 : v0 + nv].rearrange(
                            "(e p) v -> p e v", e=E_CHUNKS, p=P
                        ),
                    )
                for i, (r0, rows) in enumerate(TK_CHUNKS):
                    ps = psp.tile([P, 512], F32, tag="ps1", name="ps1", bufs=4)
                    for e in range(E_CHUNKS):
                        nc.tensor.matmul(
                            ps[:rows, :nv],
                            lhsT=topic_sb[:, e, r0 : r0 + rows],
                            rhs=wt[:, e, :nv],
                            start=(e == 0),
                            stop=(e == E_CHUNKS - 1),
                        )
                    # -chunk_rowmax (DVE), then exp-evict + chunk rowsum (ScalarE)
                    nc.vector.reduce_max(
                        negmm[i][:rows, vi : vi + 1],
                        ps[:rows, :nv],
                        axis=mybir.AxisListType.X,
                        negate=True,
                    )
                    nc.scalar.activation(
                        p_t[i][:rows, v0 : v0 + nv],
                        ps[:rows, :nv],
                        Exp,
                        bias=negmm[i][:rows, vi : vi + 1],
                        accum_out=smat[i][:rows, vi : vi + 1],
                    )

            # theta loads (phase-4 only; emitted late so startup DMA bandwidth
            # goes to topic + the first wemb slabs)
            nc.sync.dma_start(out=theta_all[:116, 3, :], in_=thetaT[384:500, :])
            nc.sync.dma_start(
                out=theta_all[:, 0:3, :],
                in_=thetaT[0:384].rearrange("(i p) b -> p i b", i=3, p=P),
            )
            # padded stat rows: m_c = 0, s_c = 1 (emitted here, used by the
            # phase-2 transpose; DVE program order keeps them before the
            # per-tile stat writes below)
            nc.vector.memset(msall[:, :, 0:1], 0.0)
            nc.vector.memset(msall[:, :, 1:2], 1.0)

            # --- phase 2: local stats + allgather ---
            for i, (r0, rows) in enumerate(TK_CHUNKS):
                # m_c = max_j m_j = -(min_j negm_j)
                nc.vector.tensor_reduce(
                    out=mrun[i][:rows, :],
                    in_=negmm[i][:rows, :nvc],
                    op=mybir.AluOpType.min,
                    axis=mybir.AxisListType.X,
                    negate=True,
                )
                nmc = const.tile([P, 1], F32, tag=f"nmc{i}", name=f"nmc{i}")
                nc.vector.tensor_scalar_mul(nmc[:rows, :], mrun[i][:rows, :], -1.0)
                # s_c = sum_j s_j * exp(m_j - m_c);  m_j = -negmm[:, j]
                wj = const.tile([P, nvc], F32, tag=f"wj{i}", name=f"wj{i}")
                nc.scalar.activation(
                    wj[:rows, :nvc],
                    negmm[i][:rows, :nvc],
                    Exp,
                    bias=nmc[:rows, :],
                    scale=-1.0,
                )
                nc.vector.tensor_mul(
                    wj[:rows, :nvc], wj[:rows, :nvc], smat[i][:rows, :nvc]
                )
                nc.vector.reduce_sum(
                    sloc[i][:rows, :], wj[:rows, :nvc], axis=mybir.AxisListType.X
                )
            # transpose [128, 8] -> [8, 128] on the (idle) PE so the stats DMA
            # is 8 contiguous 512B runs instead of a 4B-granular scatter
            mst_ps = psp.tile([8, P], F32, tag="ps2", name="mst_ps", bufs=4)
            nc.tensor.transpose(mst_ps[:], msall[:].rearrange("p i j -> p (i j)"), ident[:])
            msT = const.tile([8, P], F32, tag="msT", name="msT")
            nc.vector.tensor_copy(msT[:], mst_ps[:])
            # stats_local[0, (i*2+j)*128 + p] = m/s[tile i, row p]
            nc.sync.dma_start(
                out=stats_local[0].rearrange("(q p) -> q p", q=8, p=P), in_=msT[:]
            )
            nc.gpsimd.collective_compute(
                "AllGather",
                mybir.AluOpType.bypass,
                replica_groups=[list(range(N_CORES))],
                ins=[stats_local[:].opt()],
                outs=[stats_all[:].opt()],
            )

            # --- phase 3: global stats; per-chunk scale matrix G ---
            # natural-layout gather (8 contiguous 4KB runs), then PE-transpose
            # each [8, 128] block to the [tk-row, core] layout the combines need
            sg_all = const.tile([8, 2 * 4 * P], F32, tag="sg_all", name="sg_all")
            nc.sync.dma_start(out=sg_all[:], in_=stats_all[:])
            mst = const.tile([P, 4, 2, N_CORES], F32, tag="mst", name="mst")
            for q in range(8):
                i, j = q // 2, q % 2
                if i not in used_tiles:
                    continue
                tp = psp.tile([P, 8], F32, tag="ps2", name="mst_ps2", bufs=4)
                nc.tensor.transpose(
                    tp[:], sg_all[:, q * P : (q + 1) * P], ident[0:8, 0:8]
                )
                nc.vector.tensor_copy(mst[:, i, j, :], tp[:])
            gmat = {}
            for i in used_tiles:
                r0, rows = TK_CHUNKS[i]
                mt = mst[:, i, 0, :]
                st = mst[:, i, 1, :]
                nmg = const.tile([P, 1], F32, tag=f"nmg{i}", name=f"nmg{i}")
                nc.vector.reduce_max(
                    nmg[:], mt[:], axis=mybir.AxisListType.X, negate=True
                )
                wt = const.tile([P, N_CORES], F32, tag=f"wt{i}", name=f"wt{i}")
                nc.scalar.activation(wt[:], mt[:], Exp, bias=nmg[:])
                nc.vector.tensor_mul(wt[:], wt[:], st[:])
                sg = const.tile([P, 1], F32, tag=f"sg{i}", name=f"sg{i}")
                nc.vector.reduce_sum(sg[:], wt[:], axis=mybir.AxisListType.X)
                rg = const.tile([P, 1], F32, tag=f"rg{i}", name=f"rg{i}")
                nc.vector.reciprocal(rg[:], sg[:])
                # G[:, j] = exp(m_j - m_g) / s_g  (m_j = -negmm[:, j])
                g = const.tile([P, nvc], F32, tag=f"g{i}", name=f"g{i}")
                nc.scalar.activation(
                    g[:rows, :nvc],
                    negmm[i][:rows, :nvc],
                    Exp,
                    bias=nmg[:rows, :],
                    scale=-1.0,
                )
                nc.vector.tensor_scalar_mul(
                    g[:rows, :nvc], g[:rows, :nvc], rg[:rows, :]
                )
                gmat[i] = g

            # --- phase 4: out[b, v_j] = sum_tk theta[tk,b]*G[tk,j] * P[tk,v_j]
            # batch sorted by time on host: col block 0 needs tiles 0..bt0,
            # block 1 needs tiles bt1..3 ---
            for vi, (v0, nv) in enumerate(v_chunks):
                thv = {}
                for i in used_tiles:
                    r0, rows = TK_CHUNKS[i]
                    tv = thp.tile([P, B], _MM2_DT, tag="thv", name="thv")
                    nc.vector.tensor_scalar_mul(
                        tv[:rows, :], theta_sb[i][:rows, :], gmat[i][:rows, vi : vi + 1]
                    )
                    thv[i] = tv
                ot = opool.tile([P, 2, 512], F32, tag="ot", name="ot")
                for bi, (b0, tl) in enumerate(zip((0, P), blk_tiles)):
                    ps = psp.tile([P, 512], F32, tag="ps2", name="ps2", bufs=4)
                    for idx, i in enumerate(tl):
                        r0, rows = TK_CHUNKS[i]
                        nc.tensor.matmul(
                            ps[:, :nv],
                            lhsT=thv[i][:rows, b0 : b0 + P],
                            rhs=p_t[i][:rows, v0 : v0 + nv],
                            start=(idx == 0),
                            stop=(idx == len(tl) - 1),
                        )
                    nc.scalar.copy(ot[:, bi, :nv], ps[:, :nv])
                nc.sync.dma_start(
                    out=out[:, v0 : v0 + nv].rearrange("(b p) v -> p b v", b=2, p=P),
                    in_=ot[:, :, :nv],
                )

    nc.compile()
    return nc


_NC_CACHE = {}


def _get_nc(bt0=3, bt1=0):
    key = (bt0, bt1)
    if key not in _NC_CACHE:
        _NC_CACHE[key] = build(bt0, bt1)
    return _NC_CACHE[key]


def _plan(ti):
    """Sort batch by time slice; derive which tk-tiles each column block of
    128 needs. Any distribution is handled (worst case = dense (3, 0))."""
    perm = np.argsort(ti, kind="stable")
    tis = ti[perm]
    bt0 = int((int(tis[P - 1]) * K + K - 1) // P)
    bt1 = int((int(tis[P]) * K) // P)
    return perm, tis, bt0, bt1


def prepare(theta, word_embeddings, topic_embeddings, time_index):
    """Host-side prep shared by kernel() and the profiling harness."""
    theta = np.ascontiguousarray(np.asarray(theta), dtype=np.float32)
    wemb = np.asarray(word_embeddings, dtype=np.float32)
    topic = np.asarray(topic_embeddings, dtype=np.float32)
    ti = np.asarray(time_index).astype(np.int64)

    perm, tis, bt0, bt1 = _plan(ti)

    # time-gathered theta for the SORTED batch, transposed:
    # thetaT[t*K + k, j] = theta[perm[j], k] iff tis[j] == t
    thetaT = np.zeros((TK, B), dtype=np.float32)
    rows = (tis[:, None] * K + np.arange(K)[None, :]).ravel()
    cols = np.repeat(np.arange(B), K)
    thetaT[rows, cols] = theta[perm].ravel()

    topicT = np.ascontiguousarray(topic.reshape(TK, E).T)  # [E, TK]

    in_maps = []
    for c in range(N_CORES):
        shard = np.ascontiguousarray(wemb[c * VC : (c + 1) * VC, :].T)  # [E, VC]
        in_maps.append({"wembT": shard, "topicT": topicT, "thetaT": thetaT})

    nc = _get_nc(bt0, bt1)
    return nc, in_maps, perm


def kernel(theta, word_embeddings, topic_embeddings, time_index):
    nc, in_maps, perm = prepare(theta, word_embeddings, topic_embeddings, time_index)
    res = run_bass_kernel_spmd(nc, in_maps, core_ids=list(range(N_CORES)))
    out_sorted = np.concatenate(
        [res.results[c]["out"] for c in range(N_CORES)], axis=1
    )
    out = np.empty_like(out_sorted)
    out[perm] = out_sorted
    return out
